# revision 36
# baseline (speedup 1.0000x reference)
"""MCCA loss kernel for 8 Trainium2 NeuronCores (Bass/Tile).

Reference math (B=16384, C=395, F=2048):
  d_i        = ||x_i - centers[label_i]||^2                      (own-class distmat entry)
  loss_center= max(sum_i clip(d_i,1e-12,1e12)/B + B*(C-1)*1e-12/B, 0)
  pcmin[c]   = min_{i: label_i==c, d_i!=0} d_i   (inf if none)
  nearest    = sum_c finite(pcmin[c])
  loss_1     = max(loss_center - nearest*8/B - 8.0, 0)
  v[c]       = min_{j!=c} ||centers[c]-centers[j]||^2  (from Gram matrix)
  loss_push  = max(4500 - sum_i clip(v[label_i])/B, 0)
  out        = loss_1 + loss_push

Sharding: data-parallel over batch, 8 shards of 2048 rows; centers replicated.
Per-core partials (pcmin[512], S_center, S_val) are AllGathered on device and
reduced identically on every core; host just reads core 0's scalar output.
"""

import numpy as np
from contextlib import ExitStack

import concourse.bass as bass
import concourse.bacc as bacc
import concourse.mybir as mybir
import concourse.tile as tile
from concourse.bass_utils import run_bass_kernel_spmd
from concourse.masks import make_identity

P = 128
B, C, F = 16384, 395, 2048
NCORES = 8
SH = B // NCORES          # 2048 rows per core
NT = SH // P              # 16 item tiles per core
NQ = 4                    # class chunks (C=395 padded to 512)
CPAD = NQ * P             # 512
FK = F // P               # 16 feature chunks of 128
RHO, SIGMA = 4500.0, 8.0
BIG = 1.0e38  # "infinity" stand-in; keep BIG+BIG finite in f32 (no Inf notifications)
CLIP_LO, CLIP_HI = 1e-12, 1e12
FIN_TH = 1e37             # values >= this count as "no sample" (inf)
PSZ = 514                 # per-core partial: [0:512] pcmin, [512] S_center, [513] S_val

f32 = mybir.dt.float32
i32 = mybir.dt.int32
ALU = mybir.AluOpType
ACTF = mybir.ActivationFunctionType
AX = mybir.AxisListType


def _body(nc, tc, ctx, use_collective, stage):
    x_d = nc.dram_tensor("x", [SH, F], f32, kind="ExternalInput")
    lab_d = nc.dram_tensor("labels", [SH], i32, kind="ExternalInput")
    cen_d = nc.dram_tensor("centers", [C, F], f32, kind="ExternalInput")
    out_d = nc.dram_tensor("out", [1, 1], f32, kind="ExternalOutput")
    csq_bounce_d = nc.dram_tensor("csq_bounce", [CPAD], f32)
    partial_d = nc.dram_tensor("partial", [PSZ], f32)
    gathered_d = nc.dram_tensor("gathered", [NCORES * PSZ], f32, addr_space="Shared")

    const = ctx.enter_context(tc.tile_pool(name="const", bufs=1))
    xp = ctx.enter_context(tc.tile_pool(name="xp", bufs=3))
    cbp = ctx.enter_context(tc.tile_pool(name="cbp", bufs=3))
    dfp = ctx.enter_context(tc.tile_pool(name="dfp", bufs=2))
    ohp = ctx.enter_context(tc.tile_pool(name="ohp", bufs=2))
    mkp = ctx.enter_context(tc.tile_pool(name="mkp", bufs=2))
    tiny = ctx.enter_context(tc.tile_pool(name="tiny", bufs=2))
    wp = ctx.enter_context(tc.tile_pool(name="wp", bufs=2))
    psT = ctx.enter_context(tc.tile_pool(name="psT", bufs=2, space="PSUM"))
    psG = ctx.enter_context(tc.tile_pool(name="psG", bufs=2, space="PSUM"))
    psC = ctx.enter_context(tc.tile_pool(name="psC", bufs=2, space="PSUM"))
    psS = ctx.enter_context(tc.tile_pool(name="psS", bufs=2, space="PSUM"))

    def _probe(ap):
        nc.sync.dma_start(out_d[:], ap)

    # ---------------- constants ----------------
    ident = const.tile([P, P], f32)
    make_identity(nc, ident[:])
    ones_col = const.tile([P, 1], f32)
    nc.vector.memset(ones_col[:], 1.0)
    iota_i = const.tile([P, CPAD], i32)
    nc.gpsimd.iota(iota_i[:], pattern=[[1, CPAD]], base=0, channel_multiplier=0)
    iota_f = const.tile([P, CPAD], f32)
    nc.vector.tensor_copy(iota_f[:], iota_i[:])

    # labels: [SH] -> [P, NT] (col t = tile t's labels), plus f32 copy
    lab_sb = const.tile([P, NT], i32)
    nc.sync.dma_start(lab_sb[:], lab_d[:].rearrange("(t p) -> p t", p=P))
    lab_f = const.tile([P, NT], f32)
    nc.vector.tensor_copy(lab_f[:], lab_sb[:])

    # centers: 4 chunks of <=128 classes side by side: [P, NQ*F]
    cen_sb = const.tile([P, NQ * F], f32)
    nc.gpsimd.memset(cen_sb[:, (NQ - 1) * F:], 0.0)  # zero pad classes
    for q in range(NQ):
        r0, r1 = q * P, min(C, (q + 1) * P)
        nc.sync.dma_start(cen_sb[: r1 - r0, q * F:(q + 1) * F], cen_d[r0:r1, :])

    if stage <= 1:
        _probe(lab_f[0:1, 0:1])
        return

    # ---------------- transpose centers -> cenT[fk] = [128f, CPAD classes] ----
    cenT = []
    for fk in range(FK):
        t_ = const.tile([P, CPAD], f32, name=f"cenT{fk}")
        cenT.append(t_)
        for q in range(NQ):
            pt = psT.tile([P, P], f32, tag="pt")
            nc.tensor.transpose(pt[:], cen_sb[:, q * F + fk * P: q * F + (fk + 1) * P], ident[:])
            nc.scalar.copy(t_[:, q * P:(q + 1) * P], pt[:])

    # ---------------- c_sq (column form [P, NQ]; class = q*128+p) ------------
    csq_col = const.tile([P, NQ], f32)
    for q in range(NQ):
        nc.scalar.activation(cen_sb[:, q * F:(q + 1) * F], cen_sb[:, q * F:(q + 1) * F],
                             ACTF.Square, accum_out=csq_col[:, q:q + 1])

    if stage <= 2:
        _probe(csq_col[0:1, 0:1])
        return

    # row form via PSUM transpose + DRAM bounce; then broadcast via K=1 matmul
    pt4 = psT.tile([NQ, P], f32, tag="pt")
    nc.tensor.transpose(pt4[:], csq_col[:], ident[:])
    tmp4 = tiny.tile([NQ, P], f32)
    nc.vector.tensor_copy(tmp4[:], pt4[:])
    nc.sync.dma_start(csq_bounce_d[:].rearrange("(q f) -> q f", q=NQ), tmp4[:])
    csq_row = const.tile([1, CPAD], f32)
    nc.sync.dma_start(csq_row[:], csq_bounce_d[:].rearrange("(o k) -> o k", o=1))
    nc.vector.memset(csq_row[0:1, C:CPAD], BIG)  # pad classes excluded from min
    ones_row = const.tile([1, P], f32)
    nc.vector.memset(ones_row[:], 1.0)
    bc_ps = psG.tile([P, CPAD], f32, tag="g_ps")
    nc.tensor.matmul(bc_ps[:], ones_row[:], csq_row[:], start=True, stop=True)
    csq_bcast = const.tile([P, CPAD], f32)
    nc.vector.tensor_copy(csq_bcast[:], bc_ps[:])
    # per-partition row index (p) for diagonal masking
    rowid_i = const.tile([P, 1], i32)
    nc.gpsimd.iota(rowid_i[:], pattern=[[1, 1]], base=0, channel_multiplier=1)
    rowid_f = const.tile([P, 1], f32)
    nc.vector.tensor_copy(rowid_f[:], rowid_i[:])

    if stage <= 3:
        _probe(csq_bcast[0:1, 0:1])
        return

    # ---------------- Gram -> v[c] = clip(min_{j!=c} ||c_c - c_j||^2) --------
    v_col = const.tile([P, NQ], f32)
    for q in range(NQ):
        g_ps = psG.tile([P, CPAD], f32, tag="g_ps")
        for fk in range(FK):
            nc.tensor.matmul(g_ps[:], cenT[fk][:, q * P:(q + 1) * P], cenT[fk][:],
                             start=(fk == 0), stop=(fk == FK - 1))
        w2 = wp.tile([P, CPAD], f32)
        nc.vector.tensor_scalar(w2[:], g_ps[:], -2.0, None, op0=ALU.mult)
        nc.vector.tensor_tensor(w2[:], w2[:], csq_bcast[:], op=ALU.add)
        # exclude own class: push diagonal (j == q*128 + p) up by BIG
        rq = tiny.tile([P, 1], f32, tag="rq")
        nc.vector.tensor_scalar(rq[:], rowid_f[:], float(q * P), None, op0=ALU.add)
        diagm = wp.tile([P, CPAD], f32, tag="diagm")
        nc.vector.tensor_scalar(diagm[:], iota_f[:], rq[:], BIG,
                                op0=ALU.is_equal, op1=ALU.mult)
        nc.vector.tensor_tensor(w2[:], w2[:], diagm[:], op=ALU.add)
        vmin = tiny.tile([P, 1], f32)
        nc.vector.tensor_reduce(vmin[:], w2[:], axis=AX.X, op=ALU.min)
        nc.vector.tensor_tensor(vmin[:], vmin[:], csq_col[:, q:q + 1], op=ALU.add)
        nc.vector.tensor_scalar(v_col[:, q:q + 1], vmin[:], CLIP_LO, CLIP_HI,
                                op0=ALU.max, op1=ALU.min)

    if stage <= 4:
        _probe(v_col[0:1, 0:1])
        return

    # ---------------- stream x: d_i, counts, per-class min --------------------
    d_sb = const.tile([P, NT], f32)
    a_min = const.tile([P, CPAD], f32)
    nc.vector.memset(a_min[:], BIG)
    count_sb = const.tile([P, NQ], f32)
    nc.vector.memset(count_sb[:], 0.0)

    for t in range(NT):
        xt = xp.tile([P, F], f32)
        nc.sync.dma_start(xt[:], x_d[t * P:(t + 1) * P, :])
        cbt = cbp.tile([P, F], f32)
        nc.gpsimd.indirect_dma_start(
            out=cbt[:], out_offset=None, in_=cen_d[:],
            in_offset=bass.IndirectOffsetOnAxis(ap=lab_sb[:, t:t + 1], axis=0))
        diff = dfp.tile([P, F], f32)
        nc.vector.tensor_tensor(diff[:], xt[:], cbt[:], op=ALU.subtract)
        nc.scalar.activation(diff[:], diff[:], ACTF.Square,
                             accum_out=d_sb[:, t:t + 1])

        onehot = ohp.tile([P, CPAD], f32)
        nc.vector.tensor_scalar(onehot[:], iota_f[:], lab_f[:, t:t + 1], None,
                                op0=ALU.is_equal)
        cps = psC.tile([P, NQ], f32)
        for q in range(NQ):
            nc.tensor.matmul(cps[:, q:q + 1], onehot[:, q * P:(q + 1) * P],
                             ones_col[:], start=True, stop=True)
        nc.vector.tensor_tensor(count_sb[:], count_sb[:], cps[:], op=ALU.add)

        # d with exact zeros excluded (reference: nonzero() mask)
        eq0 = tiny.tile([P, 1], f32)
        nc.vector.tensor_scalar(eq0[:], d_sb[:, t:t + 1], 0.0, BIG,
                                op0=ALU.is_equal, op1=ALU.mult)
        dfx = tiny.tile([P, 1], f32)
        nc.vector.tensor_tensor(dfx[:], d_sb[:, t:t + 1], eq0[:], op=ALU.max)

        # masked[i,c] = onehot ? dfx_i : BIG, built exactly:
        #   t1 = onehot * dfx ; t2 = onehot*(-BIG) + BIG ; masked = t1 + t2
        t1m = mkp.tile([P, CPAD], f32, tag="t1m")
        nc.vector.tensor_scalar(t1m[:], onehot[:], dfx[:, 0:1], None, op0=ALU.mult)
        t2m = mkp.tile([P, CPAD], f32, tag="t2m")
        nc.vector.tensor_scalar(t2m[:], onehot[:], -BIG, BIG,
                                op0=ALU.mult, op1=ALU.add)
        nc.vector.tensor_tensor(t1m[:], t1m[:], t2m[:], op=ALU.add)
        nc.vector.tensor_tensor(a_min[:], a_min[:], t1m[:], op=ALU.min)

    if stage <= 5:
        _probe(d_sb[0:1, 0:1])
        return

    # ---------------- per-core partials --------------------------------------
    # S_center = sum clip(d)
    csums = tiny.tile([P, 1], f32)
    clip_d = tiny.tile([P, NT], f32)
    nc.vector.tensor_scalar(clip_d[:], d_sb[:], CLIP_LO, CLIP_HI,
                            op0=ALU.max, op1=ALU.min)
    nc.vector.tensor_reduce(csums[:], clip_d[:], axis=AX.X, op=ALU.add)
    if stage <= 60:
        _probe(csums[0:1, 0:1])
        return
    sc_ps = psS.tile([1, 1], f32, tag="scl")
    nc.tensor.matmul(sc_ps[:], csums[:], ones_col[:], start=True, stop=True)
    if stage <= 61:
        scp = tiny.tile([1, 1], f32)
        nc.vector.tensor_copy(scp[:], sc_ps[:])
        _probe(scp[0:1, 0:1])
        return

    # S_val = sum_c count[c] * v[c]
    svscr = tiny.tile([P, NQ], f32)
    svc = tiny.tile([P, 1], f32)
    nc.vector.tensor_tensor(svscr[:], count_sb[:], v_col[:], op=ALU.mult)
    nc.vector.tensor_reduce(svc[:], svscr[:], axis=AX.X, op=ALU.add)
    sv_ps = psS.tile([1, 1], f32, tag="scl")
    nc.tensor.matmul(sv_ps[:], svc[:], ones_col[:], start=True, stop=True)
    if stage <= 62:
        svp = tiny.tile([1, 1], f32)
        nc.vector.tensor_copy(svp[:], sv_ps[:])
        _probe(svp[0:1, 0:1])
        return

    # pcmin column form via PE transpose + free-dim min
    pcmin_col = tiny.tile([P, NQ], f32)
    for q in range(NQ):
        pt = psT.tile([P, P], f32, tag="pt")
        nc.tensor.transpose(pt[:], a_min[:, q * P:(q + 1) * P], ident[:])
        nc.vector.tensor_reduce(pcmin_col[:, q:q + 1], pt[:], axis=AX.X, op=ALU.min)
    if stage <= 63:
        _probe(pcmin_col[0:1, 0:1])
        return

    # pack partial vector -> DRAM
    sc_sb = tiny.tile([1, 2], f32)
    nc.vector.tensor_copy(sc_sb[0:1, 0:1], sc_ps[:])
    nc.vector.tensor_copy(sc_sb[0:1, 1:2], sv_ps[:])
    nc.sync.dma_start(partial_d[0:CPAD].rearrange("(q p) -> p q", p=P), pcmin_col[:])
    nc.sync.dma_start(partial_d[CPAD:CPAD + 2].rearrange("(o k) -> o k", o=1), sc_sb[:])

    if stage <= 6:
        _probe(sc_sb[0:1, 0:1])
        return

    # ---------------- combine across cores ------------------------------------
    if use_collective:
        nc.gpsimd.collective_compute(
            "AllGather", ALU.bypass, replica_groups=[list(range(NCORES))],
            ins=[partial_d[:]], outs=[gathered_d[:]])
    else:
        # debug: fake the gather with this core's own partial replicated
        for r in range(NCORES):
            nc.sync.dma_start(gathered_d[r * PSZ:(r + 1) * PSZ], partial_d[:])

    g_sb = const.tile([NCORES, PSZ], f32)
    nc.sync.dma_start(g_sb[:], gathered_d[:].rearrange("(r k) -> r k", r=NCORES))
    # cross-core min of pcmin chunks: transpose [8,128] -> [128,8], min along free
    comb_col = tiny.tile([P, NQ], f32)
    for q in range(NQ):
        pt = psT.tile([P, NCORES], f32, tag="pt")
        nc.tensor.transpose(pt[:], g_sb[:, q * P:(q + 1) * P], ident[0:NCORES, 0:NCORES])
        nc.vector.tensor_reduce(comb_col[:, q:q + 1], pt[:], axis=AX.X, op=ALU.min)
    # cross-core sums of the two scalars via K=8 matmuls
    sct_ps = psS.tile([1, 1], f32, tag="scl")
    nc.tensor.matmul(sct_ps[:], g_sb[:, CPAD:CPAD + 1], ones_col[0:NCORES, :],
                     start=True, stop=True)
    svt_ps = psS.tile([1, 1], f32, tag="scl")
    nc.tensor.matmul(svt_ps[:], g_sb[:, CPAD + 1:CPAD + 2], ones_col[0:NCORES, :],
                     start=True, stop=True)
    sct = tiny.tile([1, 1], f32)
    nc.vector.tensor_copy(sct[:], sct_ps[:])
    svt = tiny.tile([1, 1], f32)
    nc.vector.tensor_copy(svt[:], svt_ps[:])

    # ---------------- final scalar (identical on every core) ------------------
    fin_mask = tiny.tile([P, NQ], f32)
    nc.vector.tensor_scalar(fin_mask[:], comb_col[:], FIN_TH, None, op0=ALU.is_lt)
    fin_cap = tiny.tile([P, NQ], f32)
    nc.vector.tensor_scalar(fin_cap[:], comb_col[:], FIN_TH, None, op0=ALU.min)
    fin_scr = tiny.tile([P, NQ], f32)
    fin_row = tiny.tile([P, 1], f32)
    nc.vector.tensor_tensor(fin_scr[:], fin_mask[:], fin_cap[:], op=ALU.mult)
    nc.vector.tensor_reduce(fin_row[:], fin_scr[:], axis=AX.X, op=ALU.add)
    near_ps = psS.tile([1, 1], f32, tag="scl")
    nc.tensor.matmul(near_ps[:], fin_row[:], ones_col[:], start=True, stop=True)
    nearest_t = tiny.tile([1, 1], f32)
    nc.vector.tensor_copy(nearest_t[:], near_ps[:])

    zconst = float(B * (C - 1) * CLIP_LO)
    lc = tiny.tile([1, 1], f32)   # loss_center = max((S_center+z)/B, 0)
    nc.vector.tensor_scalar(lc[:], sct[:], zconst, 1.0 / B,
                            op0=ALU.add, op1=ALU.mult)
    nc.vector.tensor_scalar(lc[:], lc[:], 0.0, None, op0=ALU.max)
    ln = tiny.tile([1, 1], f32)   # loss_nearest = nearest*8/B
    nc.vector.tensor_scalar(ln[:], nearest_t[:], 8.0 / B, None, op0=ALU.mult)
    l1 = tiny.tile([1, 1], f32)   # max(lc - ln - sigma, 0)
    nc.vector.tensor_tensor(l1[:], lc[:], ln[:], op=ALU.subtract)
    nc.vector.tensor_scalar(l1[:], l1[:], SIGMA, 0.0, op0=ALU.subtract, op1=ALU.max)
    lp = tiny.tile([1, 1], f32)   # loss_push = max(rho - S_val/B, 0)
    nc.vector.tensor_scalar(lp[:], svt[:], -1.0 / B, RHO,
                            op0=ALU.mult, op1=ALU.add)
    nc.vector.tensor_scalar(lp[:], lp[:], 0.0, None, op0=ALU.max)
    res = tiny.tile([1, 1], f32)
    nc.vector.tensor_tensor(res[:], l1[:], lp[:], op=ALU.add)
    nc.sync.dma_start(out_d[:], res[:])


def build_nc(use_collective=True, stage=99):
    nc = bacc.Bacc(None, target_bir_lowering=False, num_devices=NCORES)
    with tile.TileContext(nc) as tc, ExitStack() as ctx:
        _body(nc, tc, ctx, use_collective, stage)
    return nc


_NC_CACHE = None


def _get_nc():
    global _NC_CACHE
    if _NC_CACHE is None:
        nc = build_nc()
        nc.finalize()  # Bacc: run compile passes (reg alloc, matmul wait split)
        _NC_CACHE = nc
    return _NC_CACHE


def make_in_maps(x, centers, labels):
    x = np.ascontiguousarray(x, dtype=np.float32)
    centers = np.ascontiguousarray(centers, dtype=np.float32)
    labels = np.ascontiguousarray(labels, dtype=np.int32)
    in_maps = []
    for i in range(NCORES):
        in_maps.append({
            "x": np.ascontiguousarray(x[i * SH:(i + 1) * SH]),
            "labels": np.ascontiguousarray(labels[i * SH:(i + 1) * SH]),
            "centers": centers,
        })
    return in_maps


def run(x, centers, labels, **kwargs):
    nc = _get_nc()
    return run_bass_kernel_spmd(nc, make_in_maps(x, centers, labels),
                                core_ids=list(range(NCORES)), **kwargs)


def kernel(x, centers, labels):
    res = run(x, centers, labels)
    return np.asarray(res.results[0]["out"].reshape(()), dtype=np.float32)


# revision 39
# speedup vs baseline: 2.1653x; 2.1653x over previous
"""MCCA loss kernel for 8 Trainium2 NeuronCores (Bass/Tile).

Reference math (B=16384, C=395, F=2048):
  d_i        = ||x_i - centers[label_i]||^2                      (own-class distmat entry)
  loss_center= max(sum_i clip(d_i,1e-12,1e12)/B + B*(C-1)*1e-12/B, 0)
  pcmin[c]   = min_{i: label_i==c, d_i!=0} d_i   (inf if none)
  nearest    = sum_c finite(pcmin[c])
  loss_1     = max(loss_center - nearest*8/B - 8.0, 0)
  v[c]       = min_{j!=c} ||centers[c]-centers[j]||^2  (from Gram matrix)
  loss_push  = max(4500 - sum_i clip(v[label_i])/B, 0)
  out        = loss_1 + loss_push

Sharding: data-parallel over batch, 8 shards of 2048 rows; centers replicated.
Per-core partials (pcmin[512], S_center, S_val) are AllGathered on device and
reduced identically on every core; host just reads core 0's scalar output.
"""

import numpy as np
from contextlib import ExitStack

import concourse.bass as bass
import concourse.bacc as bacc
import concourse.mybir as mybir
import concourse.tile as tile
from concourse.bass_utils import run_bass_kernel_spmd
from concourse.masks import make_identity

P = 128
B, C, F = 16384, 395, 2048
NCORES = 8
SH = B // NCORES          # 2048 rows per core
NT = SH // P              # 16 item tiles per core
NQ = 4                    # class chunks (C=395 padded to 512)
CPAD = NQ * P             # 512
FK = F // P               # 16 feature chunks of 128
RHO, SIGMA = 4500.0, 8.0
BIG = 1.0e38  # "infinity" stand-in; keep BIG+BIG finite in f32 (no Inf notifications)
CLIP_LO, CLIP_HI = 1e-12, 1e12
FIN_TH = 1e37             # values >= this count as "no sample" (inf)
PSZ = 514                 # per-core partial: [0:512] pcmin, [512] S_center, [513] S_val

f32 = mybir.dt.float32
i32 = mybir.dt.int32
ALU = mybir.AluOpType
ACTF = mybir.ActivationFunctionType
AX = mybir.AxisListType


def _body(nc, tc, ctx, use_collective, stage, rep=0):
    if rep == 0:
        x_d = nc.dram_tensor("x", [SH, F], f32, kind="ExternalInput")
        lab_d = nc.dram_tensor("labels", [SH], i32, kind="ExternalInput")
        cen_d = nc.dram_tensor("centers", [C, F], f32, kind="ExternalInput")
        out_d = nc.dram_tensor("out", [1, 1], f32, kind="ExternalOutput")
    else:
        x_d = bass.DRamTensorHandle("x", [SH, F], f32)
        lab_d = bass.DRamTensorHandle("labels", [SH], i32)
        cen_d = bass.DRamTensorHandle("centers", [C, F], f32)
        out_d = bass.DRamTensorHandle("out", [1, 1], f32)
    csq_bounce_d = nc.dram_tensor(f"csq_bounce{rep}", [CPAD], f32)
    partial_d = nc.dram_tensor(f"partial{rep}", [PSZ], f32)
    gathered_d = nc.dram_tensor(f"gathered{rep}", [NCORES * PSZ], f32, addr_space="Shared")

    const = ctx.enter_context(tc.tile_pool(name="const", bufs=1))
    xp = ctx.enter_context(tc.tile_pool(name="xp", bufs=3))
    cbp = ctx.enter_context(tc.tile_pool(name="cbp", bufs=3))
    dfp = ctx.enter_context(tc.tile_pool(name="dfp", bufs=2))
    ohp = ctx.enter_context(tc.tile_pool(name="ohp", bufs=2))
    mkp = ctx.enter_context(tc.tile_pool(name="mkp", bufs=2))
    tiny = ctx.enter_context(tc.tile_pool(name="tiny", bufs=2))
    wp = ctx.enter_context(tc.tile_pool(name="wp", bufs=2))
    psT = ctx.enter_context(tc.tile_pool(name="psT", bufs=2, space="PSUM"))
    psG = ctx.enter_context(tc.tile_pool(name="psG", bufs=2, space="PSUM"))
    psC = ctx.enter_context(tc.tile_pool(name="psC", bufs=2, space="PSUM"))
    psS = ctx.enter_context(tc.tile_pool(name="psS", bufs=2, space="PSUM"))

    def _probe(ap):
        nc.sync.dma_start(out_d[:], ap)

    # ---------------- constants ----------------
    ident = const.tile([P, P], f32)
    make_identity(nc, ident[:])
    ones_col = const.tile([P, 1], f32)
    nc.vector.memset(ones_col[:], 1.0)
    iota_i = const.tile([P, CPAD], i32)
    nc.gpsimd.iota(iota_i[:], pattern=[[1, CPAD]], base=0, channel_multiplier=0)
    iota_f = const.tile([P, CPAD], f32)
    nc.vector.tensor_copy(iota_f[:], iota_i[:])

    # labels: [SH] -> [P, NT] (col t = tile t's labels), plus f32 copy
    lab_sb = const.tile([P, NT], i32)
    nc.sync.dma_start(lab_sb[:], lab_d[:].rearrange("(t p) -> p t", p=P))
    lab_f = const.tile([P, NT], f32)
    nc.vector.tensor_copy(lab_f[:], lab_sb[:])

    # centers: 4 chunks of <=128 classes side by side: [P, NQ*F]
    cen_sb = const.tile([P, NQ * F], f32)
    nc.gpsimd.memset(cen_sb[:, (NQ - 1) * F:], 0.0)  # zero pad classes
    for q in range(NQ):
        r0, r1 = q * P, min(C, (q + 1) * P)
        nc.sync.dma_start(cen_sb[: r1 - r0, q * F:(q + 1) * F], cen_d[r0:r1, :])

    if stage <= 1:
        _probe(lab_f[0:1, 0:1])
        return

    # ---------------- transpose centers -> cenT[fk] = [128f, CPAD classes] ----
    cenT = []
    for fk in range(FK):
        t_ = const.tile([P, CPAD], f32, name=f"cenT{fk}")
        cenT.append(t_)
        for q in range(NQ):
            pt = psT.tile([P, P], f32, tag="pt")
            nc.tensor.transpose(pt[:], cen_sb[:, q * F + fk * P: q * F + (fk + 1) * P], ident[:])
            nc.scalar.copy(t_[:, q * P:(q + 1) * P], pt[:])

    # ---------------- c_sq (column form [P, NQ]; class = q*128+p) ------------
    csq_col = const.tile([P, NQ], f32)
    for q in range(NQ):
        nc.scalar.activation(cen_sb[:, q * F:(q + 1) * F], cen_sb[:, q * F:(q + 1) * F],
                             ACTF.Square, accum_out=csq_col[:, q:q + 1])

    if stage <= 2:
        _probe(csq_col[0:1, 0:1])
        return

    # row form via PSUM transpose + DRAM bounce; then broadcast via K=1 matmul
    pt4 = psT.tile([NQ, P], f32, tag="pt")
    nc.tensor.transpose(pt4[:], csq_col[:], ident[:])
    tmp4 = tiny.tile([NQ, P], f32)
    nc.vector.tensor_copy(tmp4[:], pt4[:])
    nc.sync.dma_start(csq_bounce_d[:].rearrange("(q f) -> q f", q=NQ), tmp4[:])
    csq_row = const.tile([1, CPAD], f32)
    nc.sync.dma_start(csq_row[:], csq_bounce_d[:].rearrange("(o k) -> o k", o=1))
    nc.vector.memset(csq_row[0:1, C:CPAD], BIG)  # pad classes excluded from min
    ones_row = const.tile([1, P], f32)
    nc.vector.memset(ones_row[:], 1.0)
    bc_ps = psG.tile([P, CPAD], f32, tag="g_ps")
    nc.tensor.matmul(bc_ps[:], ones_row[:], csq_row[:], start=True, stop=True)
    csq_bcast = const.tile([P, CPAD], f32)
    nc.vector.tensor_copy(csq_bcast[:], bc_ps[:])
    # per-partition row index (p) for diagonal masking
    rowid_i = const.tile([P, 1], i32)
    nc.gpsimd.iota(rowid_i[:], pattern=[[1, 1]], base=0, channel_multiplier=1)
    rowid_f = const.tile([P, 1], f32)
    nc.vector.tensor_copy(rowid_f[:], rowid_i[:])

    if stage <= 3:
        _probe(csq_bcast[0:1, 0:1])
        return

    # ---------------- Gram -> v[c] = clip(min_{j!=c} ||c_c - c_j||^2) --------
    v_col = const.tile([P, NQ], f32)
    for q in range(NQ):
        g_ps = psG.tile([P, CPAD], f32, tag="g_ps")
        for fk in range(FK):
            nc.tensor.matmul(g_ps[:], cenT[fk][:, q * P:(q + 1) * P], cenT[fk][:],
                             start=(fk == 0), stop=(fk == FK - 1))
        w2 = wp.tile([P, CPAD], f32)
        nc.vector.tensor_scalar(w2[:], g_ps[:], -2.0, None, op0=ALU.mult)
        nc.vector.tensor_tensor(w2[:], w2[:], csq_bcast[:], op=ALU.add)
        # exclude own class: push diagonal (j == q*128 + p) up by BIG
        rq = tiny.tile([P, 1], f32, tag="rq")
        nc.vector.tensor_scalar(rq[:], rowid_f[:], float(q * P), None, op0=ALU.add)
        diagm = wp.tile([P, CPAD], f32, tag="diagm")
        nc.vector.tensor_scalar(diagm[:], iota_f[:], rq[:], BIG,
                                op0=ALU.is_equal, op1=ALU.mult)
        nc.vector.tensor_tensor(w2[:], w2[:], diagm[:], op=ALU.add)
        vmin = tiny.tile([P, 1], f32)
        nc.vector.tensor_reduce(vmin[:], w2[:], axis=AX.X, op=ALU.min)
        nc.vector.tensor_tensor(vmin[:], vmin[:], csq_col[:, q:q + 1], op=ALU.add)
        nc.vector.tensor_scalar(v_col[:, q:q + 1], vmin[:], CLIP_LO, CLIP_HI,
                                op0=ALU.max, op1=ALU.min)

    if stage <= 4:
        _probe(v_col[0:1, 0:1])
        return

    # ---------------- stream x: d_i, counts, per-class min --------------------
    d_sb = const.tile([P, NT], f32)
    a_min = const.tile([P, CPAD], f32)
    nc.vector.memset(a_min[:], BIG)
    count_sb = const.tile([P, NQ], f32)
    nc.vector.memset(count_sb[:], 0.0)

    for t in range(NT):
        xt = xp.tile([P, F], f32)
        nc.sync.dma_start(xt[:], x_d[t * P:(t + 1) * P, :])
        cbt = cbp.tile([P, F], f32)
        nc.gpsimd.indirect_dma_start(
            out=cbt[:], out_offset=None, in_=cen_d[:],
            in_offset=bass.IndirectOffsetOnAxis(ap=lab_sb[:, t:t + 1], axis=0))
        diff = dfp.tile([P, F], f32)
        nc.vector.tensor_tensor(diff[:], xt[:], cbt[:], op=ALU.subtract)
        nc.scalar.activation(diff[:], diff[:], ACTF.Square,
                             accum_out=d_sb[:, t:t + 1])

        onehot = ohp.tile([P, CPAD], f32)
        nc.vector.tensor_scalar(onehot[:], iota_f[:], lab_f[:, t:t + 1], None,
                                op0=ALU.is_equal)
        cps = psC.tile([P, NQ], f32)
        for q in range(NQ):
            nc.tensor.matmul(cps[:, q:q + 1], onehot[:, q * P:(q + 1) * P],
                             ones_col[:], start=True, stop=True)
        nc.vector.tensor_tensor(count_sb[:], count_sb[:], cps[:], op=ALU.add)

        # d with exact zeros excluded (reference: nonzero() mask)
        eq0 = tiny.tile([P, 1], f32)
        nc.vector.tensor_scalar(eq0[:], d_sb[:, t:t + 1], 0.0, BIG,
                                op0=ALU.is_equal, op1=ALU.mult)
        dfx = tiny.tile([P, 1], f32)
        nc.vector.tensor_tensor(dfx[:], d_sb[:, t:t + 1], eq0[:], op=ALU.max)

        # masked[i,c] = onehot ? dfx_i : BIG, built exactly:
        #   t1 = onehot * dfx ; t2 = onehot*(-BIG) + BIG ; masked = t1 + t2
        t1m = mkp.tile([P, CPAD], f32, tag="t1m")
        nc.vector.tensor_scalar(t1m[:], onehot[:], dfx[:, 0:1], None, op0=ALU.mult)
        t2m = mkp.tile([P, CPAD], f32, tag="t2m")
        nc.vector.tensor_scalar(t2m[:], onehot[:], -BIG, BIG,
                                op0=ALU.mult, op1=ALU.add)
        nc.vector.tensor_tensor(t1m[:], t1m[:], t2m[:], op=ALU.add)
        nc.vector.tensor_tensor(a_min[:], a_min[:], t1m[:], op=ALU.min)

    if stage <= 5:
        _probe(d_sb[0:1, 0:1])
        return

    # ---------------- per-core partials --------------------------------------
    # S_center = sum clip(d)
    csums = tiny.tile([P, 1], f32)
    clip_d = tiny.tile([P, NT], f32)
    nc.vector.tensor_scalar(clip_d[:], d_sb[:], CLIP_LO, CLIP_HI,
                            op0=ALU.max, op1=ALU.min)
    nc.vector.tensor_reduce(csums[:], clip_d[:], axis=AX.X, op=ALU.add)
    if stage <= 60:
        _probe(csums[0:1, 0:1])
        return
    sc_ps = psS.tile([1, 1], f32, tag="scl")
    nc.tensor.matmul(sc_ps[:], csums[:], ones_col[:], start=True, stop=True)
    if stage <= 61:
        scp = tiny.tile([1, 1], f32)
        nc.vector.tensor_copy(scp[:], sc_ps[:])
        _probe(scp[0:1, 0:1])
        return

    # S_val = sum_c count[c] * v[c]
    svscr = tiny.tile([P, NQ], f32)
    svc = tiny.tile([P, 1], f32)
    nc.vector.tensor_tensor(svscr[:], count_sb[:], v_col[:], op=ALU.mult)
    nc.vector.tensor_reduce(svc[:], svscr[:], axis=AX.X, op=ALU.add)
    sv_ps = psS.tile([1, 1], f32, tag="scl")
    nc.tensor.matmul(sv_ps[:], svc[:], ones_col[:], start=True, stop=True)
    if stage <= 62:
        svp = tiny.tile([1, 1], f32)
        nc.vector.tensor_copy(svp[:], sv_ps[:])
        _probe(svp[0:1, 0:1])
        return

    # pcmin column form via PE transpose + free-dim min
    pcmin_col = tiny.tile([P, NQ], f32)
    for q in range(NQ):
        pt = psT.tile([P, P], f32, tag="pt")
        nc.tensor.transpose(pt[:], a_min[:, q * P:(q + 1) * P], ident[:])
        nc.vector.tensor_reduce(pcmin_col[:, q:q + 1], pt[:], axis=AX.X, op=ALU.min)
    if stage <= 63:
        _probe(pcmin_col[0:1, 0:1])
        return

    # pack partial vector -> DRAM
    sc_sb = tiny.tile([1, 2], f32)
    nc.vector.tensor_copy(sc_sb[0:1, 0:1], sc_ps[:])
    nc.vector.tensor_copy(sc_sb[0:1, 1:2], sv_ps[:])
    nc.sync.dma_start(partial_d[0:CPAD].rearrange("(q p) -> p q", p=P), pcmin_col[:])
    nc.sync.dma_start(partial_d[CPAD:CPAD + 2].rearrange("(o k) -> o k", o=1), sc_sb[:])

    if stage <= 6:
        _probe(sc_sb[0:1, 0:1])
        return

    # ---------------- combine across cores ------------------------------------
    if use_collective:
        nc.gpsimd.collective_compute(
            "AllGather", ALU.bypass, replica_groups=[list(range(NCORES))],
            ins=[partial_d[:]], outs=[gathered_d[:]])
    else:
        # debug: fake the gather with this core's own partial replicated
        for r in range(NCORES):
            nc.sync.dma_start(gathered_d[r * PSZ:(r + 1) * PSZ], partial_d[:])

    g_sb = const.tile([NCORES, PSZ], f32)
    nc.sync.dma_start(g_sb[:], gathered_d[:].rearrange("(r k) -> r k", r=NCORES))
    # cross-core min of pcmin chunks: transpose [8,128] -> [128,8], min along free
    comb_col = tiny.tile([P, NQ], f32)
    for q in range(NQ):
        pt = psT.tile([P, NCORES], f32, tag="pt")
        nc.tensor.transpose(pt[:], g_sb[:, q * P:(q + 1) * P], ident[0:NCORES, 0:NCORES])
        nc.vector.tensor_reduce(comb_col[:, q:q + 1], pt[:], axis=AX.X, op=ALU.min)
    # cross-core sums of the two scalars via K=8 matmuls
    sct_ps = psS.tile([1, 1], f32, tag="scl")
    nc.tensor.matmul(sct_ps[:], g_sb[:, CPAD:CPAD + 1], ones_col[0:NCORES, :],
                     start=True, stop=True)
    svt_ps = psS.tile([1, 1], f32, tag="scl")
    nc.tensor.matmul(svt_ps[:], g_sb[:, CPAD + 1:CPAD + 2], ones_col[0:NCORES, :],
                     start=True, stop=True)
    sct = tiny.tile([1, 1], f32)
    nc.vector.tensor_copy(sct[:], sct_ps[:])
    svt = tiny.tile([1, 1], f32)
    nc.vector.tensor_copy(svt[:], svt_ps[:])

    # ---------------- final scalar (identical on every core) ------------------
    fin_mask = tiny.tile([P, NQ], f32)
    nc.vector.tensor_scalar(fin_mask[:], comb_col[:], FIN_TH, None, op0=ALU.is_lt)
    fin_cap = tiny.tile([P, NQ], f32)
    nc.vector.tensor_scalar(fin_cap[:], comb_col[:], FIN_TH, None, op0=ALU.min)
    fin_scr = tiny.tile([P, NQ], f32)
    fin_row = tiny.tile([P, 1], f32)
    nc.vector.tensor_tensor(fin_scr[:], fin_mask[:], fin_cap[:], op=ALU.mult)
    nc.vector.tensor_reduce(fin_row[:], fin_scr[:], axis=AX.X, op=ALU.add)
    near_ps = psS.tile([1, 1], f32, tag="scl")
    nc.tensor.matmul(near_ps[:], fin_row[:], ones_col[:], start=True, stop=True)
    nearest_t = tiny.tile([1, 1], f32)
    nc.vector.tensor_copy(nearest_t[:], near_ps[:])

    zconst = float(B * (C - 1) * CLIP_LO)
    lc = tiny.tile([1, 1], f32)   # loss_center = max((S_center+z)/B, 0)
    nc.vector.tensor_scalar(lc[:], sct[:], zconst, 1.0 / B,
                            op0=ALU.add, op1=ALU.mult)
    nc.vector.tensor_scalar(lc[:], lc[:], 0.0, None, op0=ALU.max)
    ln = tiny.tile([1, 1], f32)   # loss_nearest = nearest*8/B
    nc.vector.tensor_scalar(ln[:], nearest_t[:], 8.0 / B, None, op0=ALU.mult)
    l1 = tiny.tile([1, 1], f32)   # max(lc - ln - sigma, 0)
    nc.vector.tensor_tensor(l1[:], lc[:], ln[:], op=ALU.subtract)
    nc.vector.tensor_scalar(l1[:], l1[:], SIGMA, 0.0, op0=ALU.subtract, op1=ALU.max)
    lp = tiny.tile([1, 1], f32)   # loss_push = max(rho - S_val/B, 0)
    nc.vector.tensor_scalar(lp[:], svt[:], -1.0 / B, RHO,
                            op0=ALU.mult, op1=ALU.add)
    nc.vector.tensor_scalar(lp[:], lp[:], 0.0, None, op0=ALU.max)
    res = tiny.tile([1, 1], f32)
    nc.vector.tensor_tensor(res[:], l1[:], lp[:], op=ALU.add)
    nc.sync.dma_start(out_d[:], res[:])


def build_nc(use_collective=True, stage=99, reps=1):
    nc = bacc.Bacc(None, target_bir_lowering=False, num_devices=NCORES)
    with tile.TileContext(nc) as tc:
        for rep in range(reps):
            with ExitStack() as ctx:
                _body(nc, tc, ctx, use_collective, stage, rep=rep)
    return nc


_NC_CACHE = None


def _get_nc():
    global _NC_CACHE
    if _NC_CACHE is None:
        nc = build_nc()
        nc.finalize()  # Bacc: run compile passes (reg alloc, matmul wait split)
        _NC_CACHE = nc
    return _NC_CACHE


def make_in_maps(x, centers, labels):
    x = np.ascontiguousarray(x, dtype=np.float32)
    centers = np.ascontiguousarray(centers, dtype=np.float32)
    labels = np.ascontiguousarray(labels, dtype=np.int32)
    in_maps = []
    for i in range(NCORES):
        in_maps.append({
            "x": np.ascontiguousarray(x[i * SH:(i + 1) * SH]),
            "labels": np.ascontiguousarray(labels[i * SH:(i + 1) * SH]),
            "centers": centers,
        })
    return in_maps


def run(x, centers, labels, **kwargs):
    nc = _get_nc()
    return run_bass_kernel_spmd(nc, make_in_maps(x, centers, labels),
                                core_ids=list(range(NCORES)), **kwargs)


def kernel(x, centers, labels):
    res = run(x, centers, labels)
    return np.asarray(res.results[0]["out"].reshape(()), dtype=np.float32)


# revision 45
# speedup vs baseline: 2.5371x; 1.1717x over previous
"""MCCA loss kernel for 8 Trainium2 NeuronCores (Bass/Tile).

Reference math (B=16384, C=395, F=2048):
  d_i        = ||x_i - centers[label_i]||^2                      (own-class distmat entry)
  loss_center= max(sum_i clip(d_i,1e-12,1e12)/B + B*(C-1)*1e-12/B, 0)
  pcmin[c]   = min_{i: label_i==c, d_i!=0} d_i   (inf if none)
  nearest    = sum_c finite(pcmin[c])
  loss_1     = max(loss_center - nearest*8/B - 8.0, 0)
  v[c]       = min_{j!=c} ||centers[c]-centers[j]||^2  (from Gram matrix)
  loss_push  = max(4500 - sum_i clip(v[label_i])/B, 0)
  out        = loss_1 + loss_push

Sharding: data-parallel over batch, 8 shards of 2048 rows; centers replicated.
Per-core partials (pcmin[512], S_center, S_val) are AllGathered on device and
reduced identically on every core; host just reads core 0's scalar output.
"""

import numpy as np
from contextlib import ExitStack

import concourse.bass as bass
import concourse.bacc as bacc
import concourse.mybir as mybir
import concourse.tile as tile
from concourse.bass_utils import run_bass_kernel_spmd
from concourse.masks import make_identity

P = 128
B, C, F = 16384, 395, 2048
NCORES = 8
SH = B // NCORES          # 2048 rows per core
NT = SH // P              # 16 item tiles per core
NQ = 4                    # class chunks (C=395 padded to 512)
CPAD = NQ * P             # 512
FK = F // P               # 16 feature chunks of 128
RHO, SIGMA = 4500.0, 8.0
BIG = 1.0e38  # "infinity" stand-in; keep BIG+BIG finite in f32 (no Inf notifications)
CLIP_LO, CLIP_HI = 1e-12, 1e12
FIN_TH = 1e5              # pcmin values >= this count as "no sample" (inf)
BIG2 = 1.0e6              # per-class-min "inf": > any real d (~4500), small enough
                          # that (d - BIG2) + BIG2 loses <0.07 absolute on d
PSZ = 514                 # per-core partial: [0:512] pcmin, [512] S_center, [513] S_val

f32 = mybir.dt.float32
i32 = mybir.dt.int32
ALU = mybir.AluOpType
ACTF = mybir.ActivationFunctionType
AX = mybir.AxisListType


def _body(nc, tc, ctx, use_collective, stage, rep=0):
    if rep == 0:
        x_d = nc.dram_tensor("x", [SH, F], f32, kind="ExternalInput")
        lab_d = nc.dram_tensor("labels", [SH], i32, kind="ExternalInput")
        cen_d = nc.dram_tensor("centers", [C, F], f32, kind="ExternalInput")
        out_d = nc.dram_tensor("out", [1, 1], f32, kind="ExternalOutput")
    else:
        x_d = bass.DRamTensorHandle("x", [SH, F], f32)
        lab_d = bass.DRamTensorHandle("labels", [SH], i32)
        cen_d = bass.DRamTensorHandle("centers", [C, F], f32)
        out_d = bass.DRamTensorHandle("out", [1, 1], f32)
    csq_bounce_d = nc.dram_tensor(f"csq_bounce{rep}", [CPAD], f32)
    partial_d = nc.dram_tensor(f"partial{rep}", [PSZ], f32)
    gathered_d = nc.dram_tensor(f"gathered{rep}", [NCORES * PSZ], f32, addr_space="Shared")

    const = ctx.enter_context(tc.tile_pool(name="const", bufs=1))
    xp = ctx.enter_context(tc.tile_pool(name="xp", bufs=3))
    cbp = ctx.enter_context(tc.tile_pool(name="cbp", bufs=3))
    dfp = ctx.enter_context(tc.tile_pool(name="dfp", bufs=2))
    ohp = ctx.enter_context(tc.tile_pool(name="ohp", bufs=2))
    mkp = ctx.enter_context(tc.tile_pool(name="mkp", bufs=2))
    tiny = ctx.enter_context(tc.tile_pool(name="tiny", bufs=2))
    wp = ctx.enter_context(tc.tile_pool(name="wp", bufs=2))
    psT = ctx.enter_context(tc.tile_pool(name="psT", bufs=2, space="PSUM"))
    psG = ctx.enter_context(tc.tile_pool(name="psG", bufs=2, space="PSUM"))
    psC = ctx.enter_context(tc.tile_pool(name="psC", bufs=2, space="PSUM"))
    psS = ctx.enter_context(tc.tile_pool(name="psS", bufs=2, space="PSUM"))

    def _probe(ap):
        nc.sync.dma_start(out_d[:], ap)

    # ---------------- constants ----------------
    ident = const.tile([P, P], f32)
    make_identity(nc, ident[:])
    ones_col = const.tile([P, 1], f32)
    nc.vector.memset(ones_col[:], 1.0)
    iota_i = const.tile([P, CPAD], i32)
    nc.gpsimd.iota(iota_i[:], pattern=[[1, CPAD]], base=0, channel_multiplier=0)
    iota_f = const.tile([P, CPAD], f32)
    nc.vector.tensor_copy(iota_f[:], iota_i[:])

    # labels: [SH] -> [P, NT] (col t = tile t's labels), plus f32 copy
    lab_sb = const.tile([P, NT], i32)
    nc.sync.dma_start(lab_sb[:], lab_d[:].rearrange("(t p) -> p t", p=P))
    lab_f = const.tile([P, NT], f32)
    nc.vector.tensor_copy(lab_f[:], lab_sb[:])

    # centers: 4 chunks of <=128 classes side by side: [P, NQ*F]
    cen_sb = const.tile([P, NQ * F], f32)
    nc.gpsimd.memset(cen_sb[:, (NQ - 1) * F:], 0.0)  # zero pad classes
    for q in range(NQ):
        r0, r1 = q * P, min(C, (q + 1) * P)
        nc.sync.dma_start(cen_sb[: r1 - r0, q * F:(q + 1) * F], cen_d[r0:r1, :])

    if stage <= 1:
        _probe(lab_f[0:1, 0:1])
        return

    # ---------------- transpose centers -> cenT[fk] = [128f, CPAD classes] ----
    # cenT is only used for the Gram matmul; store rounded to float32r so the
    # PE runs the full-rate fp32r mode (vs 1/4-rate fp32).
    cenT = []
    for fk in range(FK):
        t_ = const.tile([P, CPAD], mybir.dt.float32r, name=f"cenT{fk}")
        cenT.append(t_)
        for q in range(NQ):
            pt = psT.tile([P, P], f32, tag="pt")
            nc.tensor.transpose(pt[:], cen_sb[:, q * F + fk * P: q * F + (fk + 1) * P], ident[:])
            nc.scalar.copy(t_[:, q * P:(q + 1) * P], pt[:])

    # ---------------- c_sq (column form [P, NQ]; class = q*128+p) ------------
    csq_col = const.tile([P, NQ], f32)
    for q in range(NQ):
        nc.scalar.activation(cen_sb[:, q * F:(q + 1) * F], cen_sb[:, q * F:(q + 1) * F],
                             ACTF.Square, accum_out=csq_col[:, q:q + 1])

    if stage <= 2:
        _probe(csq_col[0:1, 0:1])
        return

    # row form via PSUM transpose + DRAM bounce; then broadcast via K=1 matmul
    pt4 = psT.tile([NQ, P], f32, tag="pt")
    nc.tensor.transpose(pt4[:], csq_col[:], ident[:])
    tmp4 = tiny.tile([NQ, P], f32)
    nc.vector.tensor_copy(tmp4[:], pt4[:])
    nc.sync.dma_start(csq_bounce_d[:].rearrange("(q f) -> q f", q=NQ), tmp4[:])
    csq_row = const.tile([1, CPAD], f32)
    nc.sync.dma_start(csq_row[:], csq_bounce_d[:].rearrange("(o k) -> o k", o=1))
    nc.vector.memset(csq_row[0:1, C:CPAD], BIG)  # pad classes excluded from min
    ones_row = const.tile([1, P], f32)
    nc.vector.memset(ones_row[:], 1.0)
    bc_ps = psG.tile([P, CPAD], f32, tag="g_ps")
    nc.tensor.matmul(bc_ps[:], ones_row[:], csq_row[:], start=True, stop=True)
    csq_bcast = const.tile([P, CPAD], f32)
    nc.vector.tensor_copy(csq_bcast[:], bc_ps[:])
    # per-partition row index (p) for diagonal masking
    rowid_i = const.tile([P, 1], i32)
    nc.gpsimd.iota(rowid_i[:], pattern=[[1, 1]], base=0, channel_multiplier=1)
    rowid_f = const.tile([P, 1], f32)
    nc.vector.tensor_copy(rowid_f[:], rowid_i[:])

    if stage <= 3:
        _probe(csq_bcast[0:1, 0:1])
        return

    # ---------------- Gram -> v[c] = clip(min_{j!=c} ||c_c - c_j||^2) --------
    # float32r: full-rate fp32 matmul mode (N=512 >= 256)
    v_col = const.tile([P, NQ], f32)
    for q in range(NQ):
        g_ps = psG.tile([P, CPAD], f32, tag="g_ps")
        for fk in range(FK):
            nc.tensor.matmul(g_ps[:], cenT[fk][:, q * P:(q + 1) * P], cenT[fk][:],
                             start=(fk == 0), stop=(fk == FK - 1))
        w2 = wp.tile([P, CPAD], f32)
        nc.vector.tensor_scalar(w2[:], g_ps[:], -2.0, None, op0=ALU.mult)
        nc.vector.tensor_tensor(w2[:], w2[:], csq_bcast[:], op=ALU.add)
        # exclude own class: push diagonal (j == q*128 + p) up by BIG
        rq = tiny.tile([P, 1], f32, tag="rq")
        nc.vector.tensor_scalar(rq[:], rowid_f[:], float(q * P), None, op0=ALU.add)
        diagm = wp.tile([P, CPAD], f32, tag="diagm")
        nc.vector.tensor_scalar(diagm[:], iota_f[:], rq[:], BIG,
                                op0=ALU.is_equal, op1=ALU.mult)
        nc.vector.tensor_tensor(w2[:], w2[:], diagm[:], op=ALU.add)
        vmin = tiny.tile([P, 1], f32)
        nc.vector.tensor_reduce(vmin[:], w2[:], axis=AX.X, op=ALU.min)
        nc.vector.tensor_tensor(vmin[:], vmin[:], csq_col[:, q:q + 1], op=ALU.add)
        nc.vector.tensor_scalar(v_col[:, q:q + 1], vmin[:], CLIP_LO, CLIP_HI,
                                op0=ALU.max, op1=ALU.min)

    if stage <= 4:
        _probe(v_col[0:1, 0:1])
        return

    # ---------------- stream x: d_i, counts, per-class min --------------------
    d_sb = const.tile([P, NT], f32)
    a_min = const.tile([P, CPAD], f32)
    nc.vector.memset(a_min[:], BIG2)
    count_sb = const.tile([P, NQ], f32)
    nc.vector.memset(count_sb[:], 0.0)

    for t in range(NT):
        xt = xp.tile([P, F], f32)
        nc.sync.dma_start(xt[:], x_d[t * P:(t + 1) * P, :])
        cbt = cbp.tile([P, F], f32)
        nc.gpsimd.indirect_dma_start(
            out=cbt[:], out_offset=None, in_=cen_d[:],
            in_offset=bass.IndirectOffsetOnAxis(ap=lab_sb[:, t:t + 1], axis=0))
        diff = dfp.tile([P, F], f32)
        nc.vector.tensor_tensor(diff[:], xt[:], cbt[:], op=ALU.subtract)
        nc.scalar.activation(diff[:], diff[:], ACTF.Square,
                             accum_out=d_sb[:, t:t + 1])

        onehot = ohp.tile([P, CPAD], f32)
        nc.vector.tensor_scalar(onehot[:], iota_f[:], lab_f[:, t:t + 1], None,
                                op0=ALU.is_equal)
        cps = psC.tile([P, NQ], f32)
        for q in range(NQ):
            nc.tensor.matmul(cps[:, q:q + 1], onehot[:, q * P:(q + 1) * P],
                             ones_col[:], start=True, stop=True)
        nc.vector.tensor_tensor(count_sb[:], count_sb[:], cps[:], op=ALU.add)

        # d with exact zeros excluded (reference: nonzero() mask)
        eq0 = tiny.tile([P, 1], f32)
        nc.vector.tensor_scalar(eq0[:], d_sb[:, t:t + 1], 0.0, BIG2,
                                op0=ALU.is_equal, op1=ALU.mult)
        dfx = tiny.tile([P, 1], f32)
        nc.vector.tensor_tensor(dfx[:], d_sb[:, t:t + 1], eq0[:], op=ALU.max)
        dmb = tiny.tile([P, 1], f32)
        nc.vector.tensor_scalar(dmb[:], dfx[:], BIG2, None, op0=ALU.subtract)

        # masked[i,c] = onehot ? ~dfx_i : BIG2   (single fused op per tile)
        t1m = mkp.tile([P, CPAD], f32, tag="t1m")
        nc.vector.tensor_scalar(t1m[:], onehot[:], dmb[:, 0:1], BIG2,
                                op0=ALU.mult, op1=ALU.add)
        nc.vector.tensor_tensor(a_min[:], a_min[:], t1m[:], op=ALU.min)

    if stage <= 5:
        _probe(d_sb[0:1, 0:1])
        return

    # ---------------- per-core partials --------------------------------------
    # S_center = sum clip(d)
    csums = tiny.tile([P, 1], f32)
    clip_d = tiny.tile([P, NT], f32)
    nc.vector.tensor_scalar(clip_d[:], d_sb[:], CLIP_LO, CLIP_HI,
                            op0=ALU.max, op1=ALU.min)
    nc.vector.tensor_reduce(csums[:], clip_d[:], axis=AX.X, op=ALU.add)
    if stage <= 60:
        _probe(csums[0:1, 0:1])
        return
    sc_ps = psS.tile([1, 1], f32, tag="scl")
    nc.tensor.matmul(sc_ps[:], csums[:], ones_col[:], start=True, stop=True)
    if stage <= 61:
        scp = tiny.tile([1, 1], f32)
        nc.vector.tensor_copy(scp[:], sc_ps[:])
        _probe(scp[0:1, 0:1])
        return

    # S_val = sum_c count[c] * v[c]
    svscr = tiny.tile([P, NQ], f32)
    svc = tiny.tile([P, 1], f32)
    nc.vector.tensor_tensor(svscr[:], count_sb[:], v_col[:], op=ALU.mult)
    nc.vector.tensor_reduce(svc[:], svscr[:], axis=AX.X, op=ALU.add)
    sv_ps = psS.tile([1, 1], f32, tag="scl")
    nc.tensor.matmul(sv_ps[:], svc[:], ones_col[:], start=True, stop=True)
    if stage <= 62:
        svp = tiny.tile([1, 1], f32)
        nc.vector.tensor_copy(svp[:], sv_ps[:])
        _probe(svp[0:1, 0:1])
        return

    # pcmin column form via PE transpose + free-dim min
    pcmin_col = tiny.tile([P, NQ], f32)
    for q in range(NQ):
        pt = psT.tile([P, P], f32, tag="pt")
        nc.tensor.transpose(pt[:], a_min[:, q * P:(q + 1) * P], ident[:])
        nc.vector.tensor_reduce(pcmin_col[:, q:q + 1], pt[:], axis=AX.X, op=ALU.min)
    if stage <= 63:
        _probe(pcmin_col[0:1, 0:1])
        return

    # pack partial vector -> DRAM
    sc_sb = tiny.tile([1, 2], f32)
    nc.vector.tensor_copy(sc_sb[0:1, 0:1], sc_ps[:])
    nc.vector.tensor_copy(sc_sb[0:1, 1:2], sv_ps[:])
    nc.sync.dma_start(partial_d[0:CPAD].rearrange("(q p) -> p q", p=P), pcmin_col[:])
    nc.sync.dma_start(partial_d[CPAD:CPAD + 2].rearrange("(o k) -> o k", o=1), sc_sb[:])

    if stage <= 6:
        _probe(sc_sb[0:1, 0:1])
        return

    # ---------------- combine across cores ------------------------------------
    if use_collective:
        nc.gpsimd.collective_compute(
            "AllGather", ALU.bypass, replica_groups=[list(range(NCORES))],
            ins=[partial_d[:]], outs=[gathered_d[:]])
    else:
        # debug: fake the gather with this core's own partial replicated
        for r in range(NCORES):
            nc.sync.dma_start(gathered_d[r * PSZ:(r + 1) * PSZ], partial_d[:])

    g_sb = const.tile([NCORES, PSZ], f32)
    nc.sync.dma_start(g_sb[:], gathered_d[:].rearrange("(r k) -> r k", r=NCORES))
    # cross-core min of pcmin chunks: transpose [8,128] -> [128,8], min along free
    comb_col = tiny.tile([P, NQ], f32)
    for q in range(NQ):
        pt = psT.tile([P, NCORES], f32, tag="pt")
        nc.tensor.transpose(pt[:], g_sb[:, q * P:(q + 1) * P], ident[0:NCORES, 0:NCORES])
        nc.vector.tensor_reduce(comb_col[:, q:q + 1], pt[:], axis=AX.X, op=ALU.min)
    # cross-core sums of the two scalars via K=8 matmuls
    sct_ps = psS.tile([1, 1], f32, tag="scl")
    nc.tensor.matmul(sct_ps[:], g_sb[:, CPAD:CPAD + 1], ones_col[0:NCORES, :],
                     start=True, stop=True)
    svt_ps = psS.tile([1, 1], f32, tag="scl")
    nc.tensor.matmul(svt_ps[:], g_sb[:, CPAD + 1:CPAD + 2], ones_col[0:NCORES, :],
                     start=True, stop=True)
    sct = tiny.tile([1, 1], f32)
    nc.vector.tensor_copy(sct[:], sct_ps[:])
    svt = tiny.tile([1, 1], f32)
    nc.vector.tensor_copy(svt[:], svt_ps[:])

    # ---------------- final scalar (identical on every core) ------------------
    fin_mask = tiny.tile([P, NQ], f32)
    nc.vector.tensor_scalar(fin_mask[:], comb_col[:], FIN_TH, None, op0=ALU.is_lt)
    fin_cap = tiny.tile([P, NQ], f32)
    nc.vector.tensor_scalar(fin_cap[:], comb_col[:], FIN_TH, None, op0=ALU.min)
    fin_scr = tiny.tile([P, NQ], f32)
    fin_row = tiny.tile([P, 1], f32)
    nc.vector.tensor_tensor(fin_scr[:], fin_mask[:], fin_cap[:], op=ALU.mult)
    nc.vector.tensor_reduce(fin_row[:], fin_scr[:], axis=AX.X, op=ALU.add)
    near_ps = psS.tile([1, 1], f32, tag="scl")
    nc.tensor.matmul(near_ps[:], fin_row[:], ones_col[:], start=True, stop=True)
    nearest_t = tiny.tile([1, 1], f32)
    nc.vector.tensor_copy(nearest_t[:], near_ps[:])

    zconst = float(B * (C - 1) * CLIP_LO)
    lc = tiny.tile([1, 1], f32)   # loss_center = max((S_center+z)/B, 0)
    nc.vector.tensor_scalar(lc[:], sct[:], zconst, 1.0 / B,
                            op0=ALU.add, op1=ALU.mult)
    nc.vector.tensor_scalar(lc[:], lc[:], 0.0, None, op0=ALU.max)
    ln = tiny.tile([1, 1], f32)   # loss_nearest = nearest*8/B
    nc.vector.tensor_scalar(ln[:], nearest_t[:], 8.0 / B, None, op0=ALU.mult)
    l1 = tiny.tile([1, 1], f32)   # max(lc - ln - sigma, 0)
    nc.vector.tensor_tensor(l1[:], lc[:], ln[:], op=ALU.subtract)
    nc.vector.tensor_scalar(l1[:], l1[:], SIGMA, 0.0, op0=ALU.subtract, op1=ALU.max)
    lp = tiny.tile([1, 1], f32)   # loss_push = max(rho - S_val/B, 0)
    nc.vector.tensor_scalar(lp[:], svt[:], -1.0 / B, RHO,
                            op0=ALU.mult, op1=ALU.add)
    nc.vector.tensor_scalar(lp[:], lp[:], 0.0, None, op0=ALU.max)
    res = tiny.tile([1, 1], f32)
    nc.vector.tensor_tensor(res[:], l1[:], lp[:], op=ALU.add)
    nc.sync.dma_start(out_d[:], res[:])


def build_nc(use_collective=True, stage=99, reps=1):
    nc = bacc.Bacc(None, target_bir_lowering=False, num_devices=NCORES)
    with tile.TileContext(nc) as tc:
        for rep in range(reps):
            with ExitStack() as ctx:
                _body(nc, tc, ctx, use_collective, stage, rep=rep)
    return nc


_NC_CACHE = None


def _get_nc():
    global _NC_CACHE
    if _NC_CACHE is None:
        nc = build_nc()
        nc.finalize()  # Bacc: run compile passes (reg alloc, matmul wait split)
        _NC_CACHE = nc
    return _NC_CACHE


def make_in_maps(x, centers, labels):
    x = np.ascontiguousarray(x, dtype=np.float32)
    centers = np.ascontiguousarray(centers, dtype=np.float32)
    labels = np.ascontiguousarray(labels, dtype=np.int32)
    in_maps = []
    for i in range(NCORES):
        in_maps.append({
            "x": np.ascontiguousarray(x[i * SH:(i + 1) * SH]),
            "labels": np.ascontiguousarray(labels[i * SH:(i + 1) * SH]),
            "centers": centers,
        })
    return in_maps


def run(x, centers, labels, **kwargs):
    nc = _get_nc()
    return run_bass_kernel_spmd(nc, make_in_maps(x, centers, labels),
                                core_ids=list(range(NCORES)), **kwargs)


def kernel(x, centers, labels):
    res = run(x, centers, labels)
    return np.asarray(res.results[0]["out"].reshape(()), dtype=np.float32)
